# revision 10
# baseline (speedup 1.0000x reference)
"""BigBird attention on 8 Trainium2 NeuronCores.

Sharding: cores 0-3 take batch 0, cores 4-7 batch 1; each core computes 3 of
the 12 heads end-to-end (q/k/v projection, masked attention, its slice of the
output projection). Host work is limited to input transposes/slices and the
final 4-way partial-sum + output bias.

Per-core dataflow (all matmuls on TensorE, f32 PSUM accumulation):
  - Augmented contraction folds the q/k/v biases in: xT_aug has a ones row at
    768, W*_aug carry the bias there (rows padded with zeros to 896 = 7*128).
  - q, k are produced transposed (head_dim on partitions); v natural.
  - Scores are computed transposed, sT[j, i] = k_j . q_i, so softmax sums come
    from an AV matmul with v augmented by a ones column: oT_u[64, i] = denom.
  - exp on ScalarE (scale folded in), mask applied as a bf16 multiply on
    VectorE (exact zeros), AV and normalization follow, then the per-head
    slice of the Wo projection accumulates 3 heads in PSUM.
"""

import sys

sys.path.insert(0, "/opt/trn_rl_repo")

import numpy as np
import ml_dtypes

import concourse.bass as bass
import concourse.tile as tile
from concourse import bacc
from concourse import mybir
from concourse.bass_utils import run_bass_kernel_spmd

B, T, D, H, HD = 2, 2048, 768, 12, 64
NCORES = 8
HPC = 3  # heads per core
DPC = HPC * HD  # 192 projected dims per core
KAUG = 896  # 768 + bias row, zero-padded to 7*128
NKT = KAUG // 128  # 7 contraction tiles
SCALE = HD ** -0.5
IT = 512  # query tile (free dim of score matmuls)
NIT = T // IT
JT = 128  # key tile (partition dim of transposed scores)
NJT = T // JT

F32 = mybir.dt.float32
F32R = mybir.dt.float32r
BF16 = mybir.dt.bfloat16

LAST_RESULTS = None  # BassKernelResults of the most recent run (for test.py)

_NC = None


def _build_nc():
    nc = bacc.Bacc(None, target_bir_lowering=False)

    xT_b = nc.declare_dram_parameter("xT_b", (KAUG, T), BF16, isOutput=False)
    wq = nc.declare_dram_parameter("wq", (KAUG, DPC), BF16, isOutput=False)
    wk = nc.declare_dram_parameter("wk", (KAUG, DPC), BF16, isOutput=False)
    wv = nc.declare_dram_parameter("wv", (KAUG, DPC), BF16, isOutput=False)
    woT = nc.declare_dram_parameter("woT", (DPC, D), F32R, isOutput=False)
    maskT = nc.declare_dram_parameter("maskT", (T, T), BF16, isOutput=False)
    y = nc.declare_dram_parameter("y", (T, D), F32, isOutput=True)

    with tile.TileContext(nc) as tc:
        _emit(nc, tc, xT_b, wq, wk, wv, woT, maskT, y)
    nc.finalize()
    return nc


def _emit(nc, tc, xT_b, wq, wk, wv, woT, maskT, y):
    import contextlib

    ctx = contextlib.ExitStack()
    with ctx:
        res = ctx.enter_context(tc.tile_pool(name="res", bufs=1))  # residents
        mpool = ctx.enter_context(tc.tile_pool(name="mask", bufs=3))
        epool = ctx.enter_context(tc.tile_pool(name="e", bufs=3))
        empool = ctx.enter_context(tc.tile_pool(name="em", bufs=3))
        opool = ctx.enter_context(tc.tile_pool(name="osb", bufs=3))
        ypool = ctx.enter_context(tc.tile_pool(name="ysb", bufs=3))
        small = ctx.enter_context(tc.tile_pool(name="small", bufs=4))

        psA = ctx.enter_context(tc.tile_pool(name="psA", bufs=4, space="PSUM"))
        psO = ctx.enter_context(tc.tile_pool(name="psO", bufs=3, space="PSUM"))
        ps1 = ctx.enter_context(tc.tile_pool(name="ps1", bufs=1, space="PSUM"))

        # ---- resident loads -------------------------------------------------
        def load_ktiled(dram, dt, free, name):
            t = res.tile([128, NKT, free], dt, name=name)
            nc.sync.dma_start(
                out=t, in_=dram.rearrange("(kt p) f -> p kt f", p=128)
            )
            return t

        xTb_sb = load_ktiled(xT_b, BF16, T, "xTb_sb")
        wq_sb = load_ktiled(wq, BF16, DPC, "wq_sb")
        wk_sb = load_ktiled(wk, BF16, DPC, "wk_sb")
        wv_sb = load_ktiled(wv, BF16, DPC, "wv_sb")
        woT_sb = res.tile([HD, HPC, D], F32R)
        nc.sync.dma_start(out=woT_sb, in_=woT.rearrange("(h d) n -> d h n", h=HPC))

        ones_f32 = res.tile([1, HD], F32)
        nc.vector.memset(ones_f32, 1.0)
        ones_col = res.tile([1, HD], F32R)
        nc.vector.tensor_copy(out=ones_col, in_=ones_f32)

        # ---- stage A: projections ------------------------------------------
        # q, k transposed: (DPC, T) as two partition groups (128 + 64)
        qT_a = res.tile([128, T], BF16)
        qT_b = res.tile([64, T], BF16)
        kT_a = res.tile([128, T], BF16)
        kT_b = res.tile([64, T], BF16)

        for w_sb, (ta, tb) in ((wq_sb, (qT_a, qT_b)), (wk_sb, (kT_a, kT_b))):
            for mt, (m0, msz, dst) in enumerate(((0, 128, ta), (128, 64, tb))):
                for nt in range(NIT):
                    ns = slice(nt * IT, (nt + 1) * IT)
                    ps = psA.tile([128, IT], F32, tag="psA", name="psqk")
                    for kt in range(NKT):
                        nc.tensor.matmul(
                            out=ps[:msz, :],
                            lhsT=w_sb[:, kt, m0 : m0 + msz],
                            rhs=xTb_sb[:, kt, ns],
                            start=(kt == 0),
                            stop=(kt == NKT - 1),
                        )
                    nc.vector.tensor_copy(out=dst[:, ns], in_=ps[:msz, :])

        # v natural, packed as [v | 1] per head: (128, NJT, HPC, 65) bf16
        vaug = res.tile([128, NJT, HPC, HD + 1], BF16)
        nc.vector.memset(vaug, 1.0)
        for jt in range(NJT):
            js = slice(jt * JT, (jt + 1) * JT)
            ps = psA.tile([128, DPC], F32, tag="psA", name="psv")
            for kt in range(NKT):
                nc.tensor.matmul(
                    out=ps,
                    lhsT=xTb_sb[:, kt, js],
                    rhs=wv_sb[:, kt, :],
                    start=(kt == 0),
                    stop=(kt == NKT - 1),
                )
            for h in range(HPC):
                nc.vector.tensor_copy(
                    out=vaug[:, jt, h, 0:HD], in_=ps[:, h * HD : (h + 1) * HD]
                )

        def qkT(tile_a, tile_b, h, fslice):
            if h < 2:
                return tile_a[h * 64 : (h + 1) * 64, fslice]
            return tile_b[0:64, fslice]

        # ---- stage B: attention --------------------------------------------
        def emit_scores(it, jt):
            isl = slice(it * IT, (it + 1) * IT)
            js = slice(jt * JT, (jt + 1) * JT)
            m_tile = mpool.tile([JT, IT], BF16, tag="mask", name="m_tile")
            nc.sync.dma_start(out=m_tile, in_=maskT[js, isl])
            sTs = []
            for h in range(HPC):
                sT = psA.tile([128, IT], F32, tag="psA", name=f"sT{h}")
                nc.tensor.matmul(
                    out=sT[:JT, :],
                    lhsT=qkT(kT_a, kT_b, h, js),
                    rhs=qkT(qT_a, qT_b, h, isl),
                    start=True,
                    stop=True,
                )
                sTs.append(sT)
            return m_tile, sTs

        pending = None  # pre-emitted (m_tile, sTs) for (it, jt=0)
        for it in range(NIT):
            oT_ps = [psO.tile([HD + 1, IT], F32, tag="psO", name=f"oT{h}") for h in range(HPC)]
            for jt in range(NJT):
                if jt == 0 and pending is not None:
                    m_tile, sTs = pending
                    pending = None
                else:
                    m_tile, sTs = emit_scores(it, jt)
                for h in range(HPC):
                    sT = sTs[h]
                    eT = epool.tile([JT, IT], BF16, tag="e", name="eT")
                    nc.scalar.activation(
                        out=eT,
                        in_=sT[:JT, :],
                        func=mybir.ActivationFunctionType.Exp,
                        scale=SCALE,
                    )
                    eTm = empool.tile([JT, IT], BF16, tag="em", name="eTm")
                    nc.vector.tensor_mul(out=eTm, in0=eT, in1=m_tile)
                    nc.tensor.matmul(
                        out=oT_ps[h],
                        lhsT=vaug[:, jt, h, :],
                        rhs=eTm,
                        start=(jt == 0),
                        stop=(jt == NJT - 1),
                    )

            # Bridge the i-tile boundary: emit the next i-tile's first score
            # matmuls NOW so the PE has dep-free work while the normalize
            # chain (reciprocal on DVE) runs — a >3us PE gap here re-throttles
            # the HAM clock for the rest of the kernel.
            if it + 1 < NIT:
                pending = emit_scores(it + 1, 0)

            oT_sb = []
            for h in range(HPC):
                recip = small.tile([1, IT], F32R, tag="recip", name="recip")
                with nc.allow_low_precision("f32r recip feeds the PE broadcast"):
                    nc.vector.reciprocal(out=recip, in_=oT_ps[h][HD : HD + 1, :])
                rb = ps1.tile([128, IT], F32, tag="ps1", name="rb")
                nc.tensor.matmul(
                    out=rb[:HD, :], lhsT=ones_col, rhs=recip, start=True, stop=True
                )
                raw = small.tile([HD, IT], F32, tag="raw", name="raw")
                nc.scalar.copy(out=raw, in_=oT_ps[h][0:HD, :])
                osb = opool.tile([HD, IT], F32R, tag="osb", name="osb")
                nc.vector.tensor_mul(out=osb, in0=raw, in1=rb[:HD, :])
                oT_sb.append(osb)

            for tb in range(IT // 128):
                t0 = it * IT + tb * 128
                ysb = ypool.tile([128, D], F32, tag="ysb", name="ysb")
                for n0, nsz in ((0, 512), (512, 256)):
                    yps = ps1.tile([128, nsz], F32, tag="ps1", name=f"yps{nsz}")
                    for h in range(HPC):
                        nc.tensor.matmul(
                            out=yps,
                            lhsT=oT_sb[h][:, tb * 128 : (tb + 1) * 128],
                            rhs=woT_sb[:, h, n0 : n0 + nsz],
                            start=(h == 0),
                            stop=(h == HPC - 1),
                        )
                    nc.scalar.copy(out=ysb[:, n0 : n0 + nsz], in_=yps)
                nc.sync.dma_start(out=y[t0 : t0 + 128, :], in_=ysb)


def _host_prep(x, Wq, bq, Wk, bk, Wv, bv, Wo, bo, mask):
    """Build the 8 per-core input maps."""
    x = np.asarray(x, dtype=np.float32)
    mask_np = np.asarray(mask)
    maskT_bf = np.ascontiguousarray(mask_np.T).astype(ml_dtypes.bfloat16)

    xTs = []
    for b in range(B):
        xa = np.zeros((KAUG, T), np.float32)
        xa[:D] = x[b].T
        xa[D] = 1.0
        xTs.append(xa)

    def w_aug(W, bias, cols):
        Wa = np.zeros((KAUG, DPC), np.float32)
        Wa[:D] = np.asarray(W, np.float32).T[:, cols]
        Wa[D] = np.asarray(bias, np.float32)[cols]
        return Wa

    in_maps = []
    for core in range(NCORES):
        b = core // 4
        h0 = HPC * (core % 4)
        cols = np.arange(h0 * HD, (h0 + HPC) * HD)
        in_maps.append(
            {
                "xT_b": xTs[b].astype(ml_dtypes.bfloat16),
                "wq": w_aug(Wq, bq, cols).astype(ml_dtypes.bfloat16),
                "wk": w_aug(Wk, bk, cols).astype(ml_dtypes.bfloat16),
                "wv": w_aug(Wv, bv, cols).astype(ml_dtypes.bfloat16),
                "woT": np.ascontiguousarray(np.asarray(Wo, np.float32).T[cols, :]),
                "maskT": maskT_bf,
            }
        )
    return in_maps


def kernel(x, Wq, bq, Wk, bk, Wv, bv, Wo, bo, mask):
    global _NC, LAST_RESULTS
    if _NC is None:
        _NC = _build_nc()

    in_maps = _host_prep(x, Wq, bq, Wk, bk, Wv, bv, Wo, bo, mask)
    res = run_bass_kernel_spmd(_NC, in_maps, list(range(NCORES)))
    LAST_RESULTS = res

    bo = np.asarray(bo, np.float32)
    out = np.zeros((B, T, D), np.float32)
    for core in range(NCORES):
        out[core // 4] += res.results[core]["y"]
    out += bo
    return out


# revision 12
# speedup vs baseline: 1.0235x; 1.0235x over previous
"""BigBird attention on 8 Trainium2 NeuronCores.

Sharding: cores 0-3 take batch 0, cores 4-7 batch 1; each core computes 3 of
the 12 heads end-to-end (q/k/v projection, masked attention, its slice of the
output projection). Host work is limited to input transposes/slices and the
final 4-way partial-sum + output bias.

Per-core dataflow (all matmuls on TensorE, f32 PSUM accumulation):
  - Augmented contraction folds the q/k/v biases in: xT_aug has a ones row at
    768, W*_aug carry the bias there (rows padded with zeros to 896 = 7*128).
  - q, k are produced transposed (head_dim on partitions); v natural.
  - Scores are computed transposed, sT[j, i] = k_j . q_i, so softmax sums come
    from an AV matmul with v augmented by a ones column: oT_u[64, i] = denom.
  - exp on ScalarE (scale folded in), mask applied as a bf16 multiply on
    VectorE (exact zeros), AV and normalization follow, then the per-head
    slice of the Wo projection accumulates 3 heads in PSUM.
"""

import sys

sys.path.insert(0, "/opt/trn_rl_repo")

import numpy as np
import ml_dtypes

import concourse.bass as bass
import concourse.tile as tile
from concourse import bacc
from concourse import mybir
from concourse.bass_utils import run_bass_kernel_spmd

B, T, D, H, HD = 2, 2048, 768, 12, 64
NCORES = 8
HPC = 3  # heads per core
DPC = HPC * HD  # 192 projected dims per core
KAUG = 896  # 768 + bias row, zero-padded to 7*128
NKT = KAUG // 128  # 7 contraction tiles
SCALE = HD ** -0.5
IT = 512  # query tile (free dim of score matmuls)
NIT = T // IT
JT = 128  # key tile (partition dim of transposed scores)
NJT = T // JT

F32 = mybir.dt.float32
F32R = mybir.dt.float32r
BF16 = mybir.dt.bfloat16

LAST_RESULTS = None  # BassKernelResults of the most recent run (for test.py)

_NC = None


def _build_nc():
    nc = bacc.Bacc(None, target_bir_lowering=False)

    xT_b = nc.declare_dram_parameter("xT_b", (KAUG, T), BF16, isOutput=False)
    wq = nc.declare_dram_parameter("wq", (KAUG, DPC), BF16, isOutput=False)
    wk = nc.declare_dram_parameter("wk", (KAUG, DPC), BF16, isOutput=False)
    wv = nc.declare_dram_parameter("wv", (KAUG, DPC), BF16, isOutput=False)
    woT = nc.declare_dram_parameter("woT", (DPC, D), F32R, isOutput=False)
    maskT = nc.declare_dram_parameter("maskT", (T, T), BF16, isOutput=False)
    y = nc.declare_dram_parameter("y", (T, D), F32, isOutput=True)

    with tile.TileContext(nc) as tc:
        _emit(nc, tc, xT_b, wq, wk, wv, woT, maskT, y)
    nc.finalize()
    return nc


def _emit(nc, tc, xT_b, wq, wk, wv, woT, maskT, y):
    import contextlib

    ctx = contextlib.ExitStack()
    with ctx:
        res = ctx.enter_context(tc.tile_pool(name="res", bufs=1))  # residents
        mpool = ctx.enter_context(tc.tile_pool(name="mask", bufs=3))
        epool = ctx.enter_context(tc.tile_pool(name="e", bufs=3))
        empool = ctx.enter_context(tc.tile_pool(name="em", bufs=3))
        opool = ctx.enter_context(tc.tile_pool(name="osb", bufs=3))
        ypool = ctx.enter_context(tc.tile_pool(name="ysb", bufs=3))
        small = ctx.enter_context(tc.tile_pool(name="small", bufs=4))

        psA = ctx.enter_context(tc.tile_pool(name="psA", bufs=3, space="PSUM"))
        psO = ctx.enter_context(tc.tile_pool(name="psO", bufs=3, space="PSUM"))
        ps1 = ctx.enter_context(tc.tile_pool(name="ps1", bufs=2, space="PSUM"))

        # ---- resident loads -------------------------------------------------
        def load_ktiled(dram, dt, free, name):
            t = res.tile([128, NKT, free], dt, name=name)
            nc.sync.dma_start(
                out=t, in_=dram.rearrange("(kt p) f -> p kt f", p=128)
            )
            return t

        xTb_sb = load_ktiled(xT_b, BF16, T, "xTb_sb")
        wq_sb = load_ktiled(wq, BF16, DPC, "wq_sb")
        wk_sb = load_ktiled(wk, BF16, DPC, "wk_sb")
        wv_sb = load_ktiled(wv, BF16, DPC, "wv_sb")
        woT_sb = res.tile([HD, HPC, D], F32R)
        nc.sync.dma_start(out=woT_sb, in_=woT.rearrange("(h d) n -> d h n", h=HPC))

        ones_f32 = res.tile([1, HD], F32)
        nc.vector.memset(ones_f32, 1.0)
        ones_col = res.tile([1, HD], F32R)
        nc.vector.tensor_copy(out=ones_col, in_=ones_f32)

        # ---- stage A: projections ------------------------------------------
        # q, k transposed: (DPC, T) as two partition groups (128 + 64)
        qT_a = res.tile([128, T], BF16)
        qT_b = res.tile([64, T], BF16)
        kT_a = res.tile([128, T], BF16)
        kT_b = res.tile([64, T], BF16)

        for w_sb, (ta, tb) in ((wq_sb, (qT_a, qT_b)), (wk_sb, (kT_a, kT_b))):
            for mt, (m0, msz, dst) in enumerate(((0, 128, ta), (128, 64, tb))):
                for nt in range(NIT):
                    ns = slice(nt * IT, (nt + 1) * IT)
                    ps = psA.tile([128, IT], F32, tag="psA", name="psqk")
                    for kt in range(NKT):
                        nc.tensor.matmul(
                            out=ps[:msz, :],
                            lhsT=w_sb[:, kt, m0 : m0 + msz],
                            rhs=xTb_sb[:, kt, ns],
                            start=(kt == 0),
                            stop=(kt == NKT - 1),
                        )
                    nc.vector.tensor_copy(out=dst[:, ns], in_=ps[:msz, :])

        # v natural, packed as [v | 1] per head: (128, NJT, HPC, 65) bf16
        vaug = res.tile([128, NJT, HPC, HD + 1], BF16)
        nc.vector.memset(vaug, 1.0)
        for jt in range(NJT):
            js = slice(jt * JT, (jt + 1) * JT)
            ps = psA.tile([128, DPC], F32, tag="psA", name="psv")
            for kt in range(NKT):
                nc.tensor.matmul(
                    out=ps,
                    lhsT=xTb_sb[:, kt, js],
                    rhs=wv_sb[:, kt, :],
                    start=(kt == 0),
                    stop=(kt == NKT - 1),
                )
            for h in range(HPC):
                nc.vector.tensor_copy(
                    out=vaug[:, jt, h, 0:HD], in_=ps[:, h * HD : (h + 1) * HD]
                )

        def qkT(tile_a, tile_b, h, fslice):
            if h < 2:
                return tile_a[h * 64 : (h + 1) * 64, fslice]
            return tile_b[0:64, fslice]

        # ---- stage B: attention --------------------------------------------
        def emit_scores(it, jt):
            isl = slice(it * IT, (it + 1) * IT)
            js = slice(jt * JT, (jt + 1) * JT)
            m_tile = mpool.tile([JT, IT], BF16, tag="mask", name="m_tile")
            nc.sync.dma_start(out=m_tile, in_=maskT[js, isl])
            sTs = []
            for h in range(HPC):
                sT = psA.tile([128, IT], F32, tag="psA", name=f"sT{h}")
                nc.tensor.matmul(
                    out=sT[:JT, :],
                    lhsT=qkT(kT_a, kT_b, h, js),
                    rhs=qkT(qT_a, qT_b, h, isl),
                    start=True,
                    stop=True,
                )
                sTs.append(sT)
            return m_tile, sTs

        pending = None  # pre-emitted (m_tile, sTs) for (it, jt=0)
        for it in range(NIT):
            oT_ps = [psO.tile([HD + 1, IT], F32, tag="psO", name=f"oT{h}") for h in range(HPC)]
            for jt in range(NJT):
                if jt == 0 and pending is not None:
                    m_tile, sTs = pending
                    pending = None
                else:
                    m_tile, sTs = emit_scores(it, jt)
                for h in range(HPC):
                    sT = sTs[h]
                    eT = epool.tile([JT, IT], BF16, tag="e", name="eT")
                    nc.scalar.activation(
                        out=eT,
                        in_=sT[:JT, :],
                        func=mybir.ActivationFunctionType.Exp,
                        scale=SCALE,
                    )
                    eTm = empool.tile([JT, IT], BF16, tag="em", name="eTm")
                    nc.vector.tensor_mul(out=eTm, in0=eT, in1=m_tile)
                    nc.tensor.matmul(
                        out=oT_ps[h],
                        lhsT=vaug[:, jt, h, :],
                        rhs=eTm,
                        start=(jt == 0),
                        stop=(jt == NJT - 1),
                    )

            # Bridge the i-tile boundary: emit the next i-tile's first score
            # matmuls NOW so the PE has dep-free work while the normalize
            # chain (reciprocal on DVE) runs — a >3us PE gap here re-throttles
            # the HAM clock for the rest of the kernel.
            if it + 1 < NIT:
                pending = emit_scores(it + 1, 0)

            oT_sb = []
            for h in range(HPC):
                # Broadcast the DENOMINATOR row via PE (dep-free except a short
                # copy), then reciprocal on all 64 partitions off the PE
                # critical path — a serial 1-partition reciprocal before the
                # broadcast stalls the PE ~3us per head and re-throttles HAM.
                dn = small.tile([1, IT], F32R, tag="dn", name="dn")
                with nc.allow_low_precision("f32r denom feeds the PE broadcast"):
                    nc.vector.tensor_copy(out=dn, in_=oT_ps[h][HD : HD + 1, :])
                rb = ps1.tile([128, IT], F32, tag="ps1", name="rb")
                nc.tensor.matmul(
                    out=rb[:HD, :], lhsT=ones_col, rhs=dn, start=True, stop=True
                )
                raw = small.tile([HD, IT], F32, tag="raw", name="raw")
                nc.scalar.copy(out=raw, in_=oT_ps[h][0:HD, :])
                rcb = small.tile([HD, IT], F32, tag="rcb", name="rcb")
                nc.vector.reciprocal(out=rcb, in_=rb[:HD, :])
                osb = opool.tile([HD, IT], F32R, tag="osb", name="osb")
                nc.vector.tensor_mul(out=osb, in0=raw, in1=rcb)
                oT_sb.append(osb)

            for tb in range(IT // 128):
                t0 = it * IT + tb * 128
                ysb = ypool.tile([128, D], F32, tag="ysb", name="ysb")
                for n0, nsz in ((0, 512), (512, 256)):
                    yps = ps1.tile([128, nsz], F32, tag="ps1", name=f"yps{nsz}")
                    for h in range(HPC):
                        nc.tensor.matmul(
                            out=yps,
                            lhsT=oT_sb[h][:, tb * 128 : (tb + 1) * 128],
                            rhs=woT_sb[:, h, n0 : n0 + nsz],
                            start=(h == 0),
                            stop=(h == HPC - 1),
                        )
                    nc.scalar.copy(out=ysb[:, n0 : n0 + nsz], in_=yps)
                nc.sync.dma_start(out=y[t0 : t0 + 128, :], in_=ysb)


def _host_prep(x, Wq, bq, Wk, bk, Wv, bv, Wo, bo, mask):
    """Build the 8 per-core input maps."""
    x = np.asarray(x, dtype=np.float32)
    mask_np = np.asarray(mask)
    maskT_bf = np.ascontiguousarray(mask_np.T).astype(ml_dtypes.bfloat16)

    xTs = []
    for b in range(B):
        xa = np.zeros((KAUG, T), np.float32)
        xa[:D] = x[b].T
        xa[D] = 1.0
        xTs.append(xa)

    def w_aug(W, bias, cols):
        Wa = np.zeros((KAUG, DPC), np.float32)
        Wa[:D] = np.asarray(W, np.float32).T[:, cols]
        Wa[D] = np.asarray(bias, np.float32)[cols]
        return Wa

    in_maps = []
    for core in range(NCORES):
        b = core // 4
        h0 = HPC * (core % 4)
        cols = np.arange(h0 * HD, (h0 + HPC) * HD)
        in_maps.append(
            {
                "xT_b": xTs[b].astype(ml_dtypes.bfloat16),
                "wq": w_aug(Wq, bq, cols).astype(ml_dtypes.bfloat16),
                "wk": w_aug(Wk, bk, cols).astype(ml_dtypes.bfloat16),
                "wv": w_aug(Wv, bv, cols).astype(ml_dtypes.bfloat16),
                "woT": np.ascontiguousarray(np.asarray(Wo, np.float32).T[cols, :]),
                "maskT": maskT_bf,
            }
        )
    return in_maps


def kernel(x, Wq, bq, Wk, bk, Wv, bv, Wo, bo, mask):
    global _NC, LAST_RESULTS
    if _NC is None:
        _NC = _build_nc()

    in_maps = _host_prep(x, Wq, bq, Wk, bk, Wv, bv, Wo, bo, mask)
    res = run_bass_kernel_spmd(_NC, in_maps, list(range(NCORES)))
    LAST_RESULTS = res

    bo = np.asarray(bo, np.float32)
    out = np.zeros((B, T, D), np.float32)
    for core in range(NCORES):
        out[core // 4] += res.results[core]["y"]
    out += bo
    return out
